# revision 4
# baseline (speedup 1.0000x reference)
"""Fused LoRA-Linear (per-token adapter routing) for 8 TRN2 NeuronCores.

Strategy:
  - Shard tokens: 8192 -> 1024 per core. Replicate weight/adapters.
  - Stack the 8 rank-16 adapters into one 128-row block:
        A_cat [128, 4096],  B_cat.T [128, 4096]
    Prologue per core: a_allT = A_cat @ x_shard^T  -> [128, 1024]  (PE)
    then ams = a_allT * smask where smask[j,t] = scal[t]*(idx[t]==j//16).
  - Main loop: out tile [128 tok, 512 dout] accumulates 32 base K-steps
    (lhsT = xT tile, rhs = W tile) plus ONE LoRA K-step
    (lhsT = ams column block, rhs = B_cat.T tile) in the same PSUM bank.
  - Drain: DVE adds broadcast bias while copying PSUM -> SBUF, DMA out.

All matmuls fp32 (PE streams 1 col/cycle regardless of dtype on TRN2, so
fp32 runs at bf16 speed; PSUM accumulates fp32 -> ~1e-6 rel err).
"""

import numpy as np

import concourse.bass as bass
import concourse.bacc as bacc
import concourse.mybir as mybir
import concourse.tile as tile
from concourse.bass_utils import run_bass_kernel_spmd

SEQ, D_IN, D_OUT, RANK, N_ADAPTERS = 8192, 4096, 4096, 16, 8
N_CORES = 8
T = SEQ // N_CORES          # 1024 tokens per core
P = 128                     # partitions
FD = 512                    # matmul free dim (fp32 max)
KO = D_IN // P              # 32 contraction tiles
NT = D_OUT // FD            # 8 output column chunks
MT = T // P                 # 8 token tiles per core
J = N_ADAPTERS * RANK       # 128 stacked adapter rows
F32 = mybir.dt.float32
MMDT = mybir.dt.float32r  # matmul operand dtype (f32r: full-rate PE, fp32 storage)

_NC_CACHE = {}


def _build_nc():
    if "nc" in _NC_CACHE:
        return _NC_CACHE["nc"]
    nc = bacc.Bacc(None, target_bir_lowering=False, debug=False)
    xT = nc.dram_tensor("xT", [D_IN, T], MMDT, kind="ExternalInput")
    w = nc.dram_tensor("w", [NT, KO, P, FD], MMDT, kind="ExternalInput")
    biasb = nc.dram_tensor("biasb", [NT, P, FD], F32, kind="ExternalInput")
    at = nc.dram_tensor("at", [KO, P, J], MMDT, kind="ExternalInput")
    bt = nc.dram_tensor("bt", [NT, J, FD], MMDT, kind="ExternalInput")
    smask = nc.dram_tensor("smask", [J, T], F32, kind="ExternalInput")
    out = nc.dram_tensor("out", [T, D_OUT], F32, kind="ExternalOutput")

    with tile.TileContext(nc) as tc:
        with (
            tc.tile_pool(name="xt", bufs=1) as xt_pool,
            tc.tile_pool(name="wp", bufs=16) as w_pool,
            tc.tile_pool(name="apool", bufs=4) as a_pool,
            tc.tile_pool(name="bp", bufs=2) as b_pool,
            tc.tile_pool(name="biasp", bufs=2) as bias_pool,
            tc.tile_pool(name="outp", bufs=4) as out_pool,
            tc.tile_pool(name="misc", bufs=1) as misc_pool,
            tc.tile_pool(name="psum", bufs=8, space="PSUM") as psum_pool,
        ):
            xT_v = xT[:].rearrange("(ko p) t -> ko p t", p=P)
            w_v = w[:]
            bias_v = biasb[:]
            at_v = at[:]
            bt_v = bt[:]
            out_v = out[:]

            # resident x^T tiles: [dk=128, 1024 tokens] each
            xts = []
            for k in range(KO):
                t_ = xt_pool.tile([P, T], MMDT, tag=f"xt{k}")
                nc.sync.dma_start(t_[:], xT_v[k])
                xts.append(t_)

            smask_sb = misc_pool.tile([J, T], F32, tag="smask")
            nc.sync.dma_start(smask_sb[:], smask[:])
            ams = misc_pool.tile([J, T], MMDT, tag="ams")

            # ---- prologue: a_allT[j, t] then mask+scale ----
            for c in range(T // FD):
                ps = psum_pool.tile([P, FD], F32, tag="ps", name=f"ps_pro_{c}")
                for k in range(KO):
                    a_sb = a_pool.tile([P, J], MMDT, tag="a")
                    nc.sync.dma_start(a_sb[:], at_v[k])
                    nc.tensor.matmul(
                        ps[:], a_sb[:], xts[k][:, c * FD:(c + 1) * FD],
                        start=(k == 0), stop=(k == KO - 1),
                    )
                nc.vector.tensor_mul(
                    out=ams[:, c * FD:(c + 1) * FD],
                    in0=ps[:],
                    in1=smask_sb[:, c * FD:(c + 1) * FD],
                )

            # ---- main: base GEMM + fused LoRA-B step ----
            for n in range(NT):
                b_sb = b_pool.tile([J, FD], MMDT, tag="b")
                nc.sync.dma_start(b_sb[:], bt_v[n])
                bias_sb = bias_pool.tile([P, FD], F32, tag="bias")
                nc.sync.dma_start(bias_sb[:], bias_v[n])
                pss = [psum_pool.tile([P, FD], F32, tag="ps", name=f"ps_{n}_{i}") for i in range(MT)]
                for k in range(KO):
                    w_sb = w_pool.tile([P, FD], MMDT, tag="w")
                    nc.sync.dma_start(w_sb[:], w_v[n, k])
                    for m in range(MT):
                        nc.tensor.matmul(
                            pss[m][:], xts[k][:, m * P:(m + 1) * P], w_sb[:],
                            start=(k == 0), stop=False,
                        )
                for m in range(MT):
                    nc.tensor.matmul(
                        pss[m][:], ams[:, m * P:(m + 1) * P], b_sb[:],
                        start=False, stop=True,
                    )
                    o_sb = out_pool.tile([P, FD], F32, tag="o")
                    nc.vector.tensor_add(out=o_sb[:], in0=pss[m][:], in1=bias_sb[:])
                    nc.sync.dma_start(
                        out_v[m * P:(m + 1) * P, n * FD:(n + 1) * FD], o_sb[:]
                    )

    nc.compile()
    _NC_CACHE["nc"] = nc
    return nc


def _prep_in_maps(x, weight, bias, A_buffer, B_buffer, scalings, token_indices):
    x = np.ascontiguousarray(np.asarray(x, np.float32))
    weight = np.asarray(weight, np.float32)
    bias = np.asarray(bias, np.float32)
    A_buffer = np.asarray(A_buffer, np.float32)
    B_buffer = np.asarray(B_buffer, np.float32)
    scalings = np.asarray(scalings, np.float32)
    token_indices = np.asarray(token_indices)

    xT_full = np.ascontiguousarray(x.T)  # [D_IN, SEQ]
    w_t = np.ascontiguousarray(
        weight.reshape(KO, P, NT, FD).transpose(2, 0, 1, 3)
    )  # [NT, KO, P, FD]
    biasb = np.ascontiguousarray(
        np.broadcast_to(bias.reshape(NT, FD)[:, None, :], (NT, P, FD))
    )
    A_cat = A_buffer.reshape(J, D_IN)
    at = np.ascontiguousarray(A_cat.T.reshape(KO, P, J))
    bt = np.ascontiguousarray(
        B_buffer.transpose(0, 2, 1).reshape(J, NT, FD).transpose(1, 0, 2)
    )  # [NT, J, FD]
    adapter_of_row = (np.arange(J) // RANK).astype(token_indices.dtype)
    smask_full = (
        (token_indices[None, :] == adapter_of_row[:, None]).astype(np.float32)
        * scalings[None, :]
    )  # [J, SEQ]

    in_maps = []
    for c in range(N_CORES):
        sl = slice(c * T, (c + 1) * T)
        in_maps.append({
            "xT": np.ascontiguousarray(xT_full[:, sl]),
            "w": w_t,
            "biasb": biasb,
            "at": at,
            "bt": bt,
            "smask": np.ascontiguousarray(smask_full[:, sl]),
        })
    return in_maps


def _run(inputs, trace=False):
    nc = _build_nc()
    in_maps = _prep_in_maps(**inputs)
    res = run_bass_kernel_spmd(
        nc, in_maps, core_ids=list(range(N_CORES)), trace=trace
    )
    out = np.concatenate([r["out"] for r in res.results], axis=0)
    return out, res


def kernel(**inputs) -> np.ndarray:
    out, _ = _run(inputs, trace=False)
    return out


# revision 21
# speedup vs baseline: 38.1555x; 38.1555x over previous
"""Fused LoRA-Linear (per-token adapter routing) for 8 TRN2 NeuronCores.

Strategy:
  - Shard tokens: 8192 -> 1024 per core. Replicate weight/adapters.
  - Stack the 8 rank-16 adapters into one 128-row block:
        A_cat [128, 4096],  B_cat.T [128, 4096]
    Prologue per core: a_allT = A_cat @ x_shard^T  -> [128, 1024]  (PE)
    then ams = a_allT * smask where smask[j,t] = scal[t]*(idx[t]==j//16).
  - Main loop: out tile [128 tok, 512 dout] accumulates 32 base K-steps
    (lhsT = xT tile, rhs = W tile) plus ONE LoRA K-step
    (lhsT = ams column block, rhs = B_cat.T tile) in the same PSUM bank.
  - Drain: DVE adds broadcast bias while copying PSUM -> SBUF, DMA out.
  - n=0 fuses the adapter prologue into its k-loop so the 16MB x^T load
    streams concurrently with base matmuls (PSUM: 2 prologue + 6 base
    banks, then m=6,7 in a second sweep).

Matmul operands are float32r (fp32 storage, reduced-precision multiply,
fp32 PSUM accumulate): 1 PE cycle/row vs 4 for strict fp32 -> ~4x faster,
measured 1.6e-4 frobenius rel err vs the fp32 reference.
"""

import numpy as np

import concourse.bass as bass
import concourse.bacc as bacc
import concourse.mybir as mybir
import concourse.tile as tile
from concourse.bass_utils import run_bass_kernel_spmd

SEQ, D_IN, D_OUT, RANK, N_ADAPTERS = 8192, 4096, 4096, 16, 8
N_CORES = 8
T = SEQ // N_CORES          # 1024 tokens per core
P = 128                     # partitions
FD = 512                    # matmul free dim (fp32 max)
KO = D_IN // P              # 32 contraction tiles
NT = D_OUT // FD            # 8 output column chunks
MT = T // P                 # 8 token tiles per core
J = N_ADAPTERS * RANK       # 128 stacked adapter rows
F32 = mybir.dt.float32
MMDT = mybir.dt.float32r  # matmul operand dtype (f32r: full-rate PE, fp32 storage)

_NC_CACHE = {}


def _build_nc():
    if "nc" in _NC_CACHE:
        return _NC_CACHE["nc"]
    nc = bacc.Bacc(None, target_bir_lowering=False, debug=False)
    xT = nc.dram_tensor("xT", [D_IN, T], MMDT, kind="ExternalInput")
    w = nc.dram_tensor("w", [NT, KO, P, FD], MMDT, kind="ExternalInput")
    biasb = nc.dram_tensor("biasb", [NT, P, FD], F32, kind="ExternalInput")
    at = nc.dram_tensor("at", [KO, P, J], MMDT, kind="ExternalInput")
    bt = nc.dram_tensor("bt", [NT, J, FD], MMDT, kind="ExternalInput")
    smask = nc.dram_tensor("smask", [J, T], F32, kind="ExternalInput")
    out = nc.dram_tensor("out", [T, D_OUT], F32, kind="ExternalOutput")

    with tile.TileContext(nc) as tc:
        with (
            tc.tile_pool(name="xt", bufs=1) as xt_pool,
            tc.tile_pool(name="wp", bufs=16) as w_pool,
            tc.tile_pool(name="apool", bufs=2) as a_pool,
            tc.tile_pool(name="bp", bufs=2) as b_pool,
            tc.tile_pool(name="biasp", bufs=1) as bias_pool,
            tc.tile_pool(name="outp", bufs=8) as out_pool,
            tc.tile_pool(name="misc", bufs=1) as misc_pool,
            tc.tile_pool(name="psum", bufs=8, space="PSUM") as psum_pool,
        ):
            xT_v = xT[:].rearrange("(ko p) t -> ko p t", p=P)
            w_v = w[:]
            bias_v = biasb[:]
            at_v = at[:]
            bt_v = bt[:]
            out_v = out[:]

            # resident x^T tiles, DMA'd inside the n=0 loop as consumed
            xts = [None] * KO

            smask_sb = misc_pool.tile([J, T], F32, tag="smask")
            nc.sync.dma_start(smask_sb[:], smask[:])
            ams = misc_pool.tile([J, T], MMDT, tag="ams")

            NCH = T // FD  # a_allT token chunks (2)
            psa = [None] * NCH

            # n=0 splits m into (0..5)+(6,7): the 2 a_allT PSUM banks + 6
            # base banks fill PSUM during the first k-sweep.
            for n in range(NT):
                groups = [range(6), range(6, MT)] if n == 0 else [range(MT)]
                b_sb = b_pool.tile([J, FD], MMDT, tag="b")
                nc.sync.dma_start(b_sb[:], bt_v[n])
                bias_sb = bias_pool.tile([P, FD], F32, tag="bias")
                nc.sync.dma_start(bias_sb[:], bias_v[n])
                for gi, ms in enumerate(groups):
                    fuse_pro = (n == 0 and gi == 0)
                    if fuse_pro:
                        for c in range(NCH):
                            psa[c] = psum_pool.tile(
                                [P, FD], F32, tag="ps", name=f"psa_{c}"
                            )
                    pss = {
                        m: psum_pool.tile([P, FD], F32, tag="ps", name=f"ps_{n}_{m}")
                        for m in ms
                    }
                    for k in range(KO):
                        last_k = k == KO - 1
                        if n == 0 and gi == 0:
                            xts[k] = xt_pool.tile(
                                [P, T], MMDT, tag=f"xt{k}", name=f"xt{k}"
                            )
                            nc.sync.dma_start(xts[k][:], xT_v[k])
                        w_sb = w_pool.tile([P, FD], MMDT, tag="w")
                        nc.sync.dma_start(w_sb[:], w_v[n, k])
                        if fuse_pro:
                            a_sb = a_pool.tile([P, J], MMDT, tag="a")
                            nc.sync.dma_start(a_sb[:], at_v[k])
                            for c in range(NCH):
                                nc.tensor.matmul(
                                    psa[c][:], a_sb[:],
                                    xts[k][:, c * FD:(c + 1) * FD],
                                    start=(k == 0), stop=last_k,
                                )
                        if fuse_pro and last_k:
                            for c in range(NCH):
                                nc.vector.tensor_mul(
                                    out=ams[:, c * FD:(c + 1) * FD],
                                    in0=psa[c][:],
                                    in1=smask_sb[:, c * FD:(c + 1) * FD],
                                )
                        for m in ms:
                            nc.tensor.matmul(
                                pss[m][:], xts[k][:, m * P:(m + 1) * P], w_sb[:],
                                start=(k == 0), stop=False,
                            )
                            if last_k:
                                # fused LoRA step + early staggered drain
                                nc.tensor.matmul(
                                    pss[m][:], ams[:, m * P:(m + 1) * P], b_sb[:],
                                    start=False, stop=True,
                                )
                                o_sb = out_pool.tile([P, FD], F32, tag="o")
                                nc.vector.tensor_add(
                                    out=o_sb[:], in0=pss[m][:], in1=bias_sb[:]
                                )
                                nc.sync.dma_start(
                                    out_v[m * P:(m + 1) * P, n * FD:(n + 1) * FD],
                                    o_sb[:],
                                )

    nc.compile()
    _NC_CACHE["nc"] = nc
    return nc


def _prep_in_maps(x, weight, bias, A_buffer, B_buffer, scalings, token_indices):
    x = np.ascontiguousarray(np.asarray(x, np.float32))
    weight = np.asarray(weight, np.float32)
    bias = np.asarray(bias, np.float32)
    A_buffer = np.asarray(A_buffer, np.float32)
    B_buffer = np.asarray(B_buffer, np.float32)
    scalings = np.asarray(scalings, np.float32)
    token_indices = np.asarray(token_indices)

    xT_full = np.ascontiguousarray(x.T)  # [D_IN, SEQ]
    w_t = np.ascontiguousarray(
        weight.reshape(KO, P, NT, FD).transpose(2, 0, 1, 3)
    )  # [NT, KO, P, FD]
    biasb = np.ascontiguousarray(
        np.broadcast_to(bias.reshape(NT, FD)[:, None, :], (NT, P, FD))
    )
    A_cat = A_buffer.reshape(J, D_IN)
    at = np.ascontiguousarray(A_cat.T.reshape(KO, P, J))
    bt = np.ascontiguousarray(
        B_buffer.transpose(0, 2, 1).reshape(J, NT, FD).transpose(1, 0, 2)
    )  # [NT, J, FD]
    adapter_of_row = (np.arange(J) // RANK).astype(token_indices.dtype)
    smask_full = (
        (token_indices[None, :] == adapter_of_row[:, None]).astype(np.float32)
        * scalings[None, :]
    )  # [J, SEQ]

    in_maps = []
    for c in range(N_CORES):
        sl = slice(c * T, (c + 1) * T)
        in_maps.append({
            "xT": np.ascontiguousarray(xT_full[:, sl]),
            "w": w_t,
            "biasb": biasb,
            "at": at,
            "bt": bt,
            "smask": np.ascontiguousarray(smask_full[:, sl]),
        })
    return in_maps


def _run(inputs, trace=False):
    nc = _build_nc()
    in_maps = _prep_in_maps(**inputs)
    res = run_bass_kernel_spmd(
        nc, in_maps, core_ids=list(range(N_CORES)), trace=trace
    )
    out = np.concatenate([r["out"] for r in res.results], axis=0)
    return out, res


def kernel(**inputs) -> np.ndarray:
    out, _ = _run(inputs, trace=False)
    return out
